# revision 1
# baseline (speedup 1.0000x reference)
"""Trainium2 Bass kernel for nn_NeuromorphicNetwork (8-core SPMD).

Math: with REFRACT=1.0 and current_time = spike_count, after a neuron's first
spike (last=t0, count=t0+1) the gate `t - last > 1` evaluates to exactly 1 > 1
= False forever, so every neuron spikes AT MOST ONCE over the entire batch
scan, and threshold adaptation (count>10) never triggers.  Pre-first-spike the
membrane follows the unreset linear recurrence; over one batch item (10 steps,
constant current c): v' = lam^10 * v + g10 * c with g10 = (1-lam^10)/(1-lam),
and a spike occurs within the item iff v' >= 1 (the 10-step trajectory is
monotone between endpoints, entry v < 1).  So per neuron the whole scan
reduces to: w_b = alpha*w_{b-1} + d_b (d = raw current), first b with
w_b >= THR -> one-hot spike-rate row of value 0.1.

Sharding (hint): tensor-parallel over hidden_dim; rate encoding sharded over
input_dim with an AllGather of spike counts (uint8); output currents
ReduceScatter(add) over the output dim; batch stays a free dim everywhere.

Per-core SPMD program (core m):
  stage A: counts[i,b] = #{t: u[b,i,t] < sigmoid(x[b,i])} for its 512-wide
           input-dim slice (uniforms are key-42 constants, shipped bf16)
  AllGather counts (uint8, 2.1MB) -> full [4096, 512]
  GEMM1  : cur_hT[h,b] = W_ih[:, h-slice].T @ counts   (fp32r matmuls)
  scan   : w = scan(alpha, cur_hT) along b; first crossing -> fT[h,b] one-hot
  GEMM2  : cur_oT10[o,b] = W_ho[h-slice,:].T @ fT      (partial over hidden)
  ReduceScatter(add, bf16) -> this core's 128-row output slice
  scan   : same first-crossing on output layer -> 0.1 * one-hot -> out [128,512]
Host assembles out[b, o] from the 8 transposed slices.
"""

import sys
import numpy as np

for _p in ("/opt/trn_rl_repo", "/root/.axon_site/_ro/trn_rl_repo"):
    if _p not in sys.path:
        sys.path.insert(0, _p)

import concourse.bass as bass
import concourse.mybir as mybir
import concourse.tile as tile
from concourse.tile_rust import add_dep_helper
from concourse import bacc
from concourse.bass_utils import run_bass_kernel_spmd

F32 = mybir.dt.float32
F32R = mybir.dt.float32r
BF16 = mybir.dt.bfloat16
U8 = mybir.dt.uint8
I32 = mybir.dt.int32
AL = mybir.AluOpType
ACT = mybir.ActivationFunctionType

B = 512            # batch (free dim everywhere)
IN_DIM = 4096
HID = 8192
OUT = 1024
T = 10
NCORES = 8
IN_SL = IN_DIM // NCORES    # 512 input dims per core
H_SL = HID // NCORES        # 1024 hidden per core
O_SL = OUT // NCORES        # 128 outputs per core
P = 128

# exact scalar constants (float64 derivation, float32 use)
_LAM = np.float64(np.float32(0.95))
ALPHA = float(_LAM ** 10)                                # per-item decay
_G10 = float((1.0 - _LAM ** 10) / (1.0 - _LAM))          # per-item current gain
# true v = 0.1 * G10 * w  (w is the scan of raw count-currents);  v >= 1  <=>  w >= THR
THR = float(10.0 / _G10)
BIGB = 1024.0      # > any valid batch index sentinel offset


def _build_nc():
    nc = bacc.Bacc(num_devices=NCORES)

    xt = nc.declare_dram_parameter("xt", [IN_SL, B], F32, isOutput=False)
    u = nc.declare_dram_parameter("u", [IN_SL // P, P, T, B], BF16, isOutput=False)
    w_ih = nc.declare_dram_parameter("w_ih", [IN_DIM, H_SL], BF16, isOutput=False)
    w_ho = nc.declare_dram_parameter("w_ho", [H_SL, OUT], BF16, isOutput=False)
    res = nc.declare_dram_parameter("res", [O_SL, B], F32, isOutput=True)

    iota_np = np.broadcast_to(np.arange(B, dtype=np.float32), (P, B))
    iota_dram = nc.inline_tensor(np.ascontiguousarray(iota_np), name="iota_c")

    with tile.TileContext(nc, num_cores=NCORES) as tc:
        with (
            tc.tile_pool(name="const", bufs=1) as constp,
            tc.tile_pool(name="dram", bufs=1, space="DRAM") as dramp,
            tc.tile_pool(name="stgA", bufs=2) as apool,
            tc.tile_pool(name="ubuf", bufs=2) as upool,
            tc.tile_pool(name="wih", bufs=4) as wpool,
            tc.tile_pool(name="rate", bufs=3) as rpool,
            tc.tile_pool(name="fT", bufs=8) as fpool,
            tc.tile_pool(name="who", bufs=8) as wopool,
            tc.tile_pool(name="scan", bufs=3) as spool,
            tc.tile_pool(name="outb", bufs=2) as obpool,
        ):
            # ---- constants ----
            iota_f = constp.tile([P, B], F32, name="iota_f")
            nc.sync.dma_start(iota_f, iota_dram[:, :])
            iota_hi = constp.tile([P, B], F32, name="iota_hi")
            nc.vector.tensor_scalar_add(iota_hi, iota_f, BIGB)
            alpha_t = constp.tile([P, B], F32, name="alpha_t")
            nc.vector.memset(alpha_t, ALPHA)

            # ---- stage A: spike-count encoding on this core's input slice ----
            # all-bf16 packed SBUF operands so the DVE 2x/4x perf modes engage
            # (broadcast APs and TensorReduce run at 1 elem/cycle).
            cnt_local = dramp.tile([IN_SL, B], U8, name="cnt_local")
            u_dmas = []
            for p in range(IN_SL // P):
                xt_sb = apool.tile([P, B], F32, name="xt_sb", tag="xt")
                nc.sync.dma_start(xt_sb, xt[p * P:(p + 1) * P, :])
                sig = apool.tile([P, B], BF16, name="sig", tag="sig")
                nc.scalar.activation(sig, xt_sb, ACT.Sigmoid)
                u_sb = upool.tile([P, T * B], BF16, name="u_sb", tag="u", bufs=4)
                u_dmas.append(nc.sync.dma_start(u_sb, u[p].rearrange("p t b -> p (t b)")))
                # replicate sig across the 10 t-planes by doubling copies
                rep = upool.tile([P, T * B], BF16, name="rep", tag="rep")
                nc.vector.tensor_copy(rep[:, 0:B], sig)
                nc.vector.tensor_copy(rep[:, B:2 * B], rep[:, 0:B])
                nc.vector.tensor_copy(rep[:, 2 * B:4 * B], rep[:, 0:2 * B])
                nc.vector.tensor_copy(rep[:, 4 * B:8 * B], rep[:, 0:4 * B])
                nc.vector.tensor_copy(rep[:, 8 * B:10 * B], rep[:, 0:2 * B])
                cmp = upool.tile([P, T * B], BF16, name="cmp", tag="cmp")
                nc.vector.tensor_tensor(cmp, u_sb, rep, AL.is_lt)
                # tree-sum the 10 t-planes (integers <= 10, exact in bf16)
                s1 = apool.tile([P, 5 * B], BF16, name="s1", tag="s1")
                nc.vector.tensor_tensor(s1, cmp[:, :5 * B], cmp[:, 5 * B:], AL.add)
                s2 = apool.tile([P, 2 * B], BF16, name="s2", tag="s2")
                nc.vector.tensor_tensor(s2, s1[:, :2 * B], s1[:, 2 * B:4 * B], AL.add)
                s3 = apool.tile([P, B], BF16, name="s3", tag="s3")
                nc.vector.tensor_tensor(s3, s2[:, :B], s2[:, B:], AL.add)
                cnt_bf = apool.tile([P, B], BF16, name="cnt_bf", tag="cntbf")
                nc.vector.tensor_tensor(cnt_bf, s3, s1[:, 4 * B:], AL.add)
                cnt8 = apool.tile([P, B], U8, name="cnt8", tag="cnt8")
                nc.vector.tensor_copy(cnt8, cnt_bf)
                nc.gpsimd.dma_start(cnt_local[p * P:(p + 1) * P, :], cnt8)

            # ---- AllGather counts across cores ----
            cnt_all = dramp.tile([IN_DIM, B], U8, name="cnt_all", addr_space="Shared")
            ag_inst = nc.gpsimd.collective_compute(
                "AllGather", AL.bypass,
                replica_groups=[list(range(NCORES))],
                ins=[cnt_local[:, :]], outs=[cnt_all[:, :]],
            )

            # ---- GEMM1: cur_hT[h, b] = W_ih[:, h-slice].T @ counts ----
            with tc.tile_pool(name="psh", bufs=8, space="PSUM") as pshp:
                psum_h = [pshp.tile([P, B], F32, name=f"ph{m}", tag="ph")
                          for m in range(H_SL // P)]
                KT = IN_DIM // P  # 32
                for k in range(KT):
                    cnt_sb = rpool.tile([P, B], U8, name="cnt_sb", tag="cntsb")
                    nc.sync.dma_start(cnt_sb, cnt_all[k * P:(k + 1) * P, :])
                    rate = rpool.tile([P, B], BF16, name="rate", tag="rate")
                    nc.vector.tensor_copy(rate, cnt_sb)
                    w_sb = wpool.tile([P, H_SL], BF16, name="w_sb", tag="wih")
                    nc.sync.dma_start(w_sb, w_ih[k * P:(k + 1) * P, :])
                    for m in range(H_SL // P):
                        nc.tensor.matmul(
                            psum_h[m],
                            lhsT=w_sb[:, m * P:(m + 1) * P],
                            rhs=rate,
                            start=(k == 0), stop=(k == KT - 1),
                        )

                # ---- hidden layer: filter scan + first-crossing one-hot ----
                fT = [fpool.tile([P, B], BF16, name=f"fT{m}", tag="fT")
                      for m in range(H_SL // P)]
                for m in range(H_SL // P):
                    w_scan = spool.tile([P, B], F32, name="w_scan", tag="wscan")
                    nc.vector.tensor_tensor_scan(
                        w_scan, alpha_t, psum_h[m], 0.0, AL.mult, AL.add)
                    g = spool.tile([P, B], F32, name="g", tag="g")
                    nc.vector.tensor_scalar(g, w_scan, THR, None, AL.is_ge)
                    midx = spool.tile([P, B], F32, name="midx", tag="midx")
                    nc.vector.scalar_tensor_tensor(
                        midx, g, -BIGB, iota_hi, AL.mult, AL.add)
                    bmin = spool.tile([P, 1], F32, name="bmin", tag="bmin")
                    nc.vector.tensor_reduce(
                        bmin, midx, axis=mybir.AxisListType.X, op=AL.min)
                    nc.vector.tensor_scalar(fT[m], iota_f, bmin, None, AL.is_equal)

            # ---- GEMM2: partial output currents over this hidden slice ----
            rs_in = dramp.tile([OUT, B], BF16, name="rs_in")
            with tc.tile_pool(name="pso", bufs=8, space="PSUM") as psop:
                psum_o = [psop.tile([P, B], F32, name=f"po{o}", tag="po")
                          for o in range(OUT // P)]
                MT = H_SL // P  # 8
                who_sbs = []
                for m in range(MT):
                    who_sb = wopool.tile([P, OUT], BF16, name=f"who{m}", tag="who")
                    wd = nc.sync.dma_start(who_sb, w_ho[m * P:(m + 1) * P, :])
                    if m == 0:
                        add_dep_helper(wd.ins, ag_inst.ins, sync=True,
                                       reason="W_ho not needed until GEMM2; keep queues clear")
                    who_sbs.append(who_sb)
                for m in range(MT - 1):
                    for o in range(OUT // P):
                        nc.tensor.matmul(
                            psum_o[o],
                            lhsT=who_sbs[m][:, o * P:(o + 1) * P],
                            rhs=fT[m],
                            start=(m == 0), stop=False,
                        )
                for o in range(OUT // P):
                    nc.tensor.matmul(
                        psum_o[o],
                        lhsT=who_sbs[MT - 1][:, o * P:(o + 1) * P],
                        rhs=fT[MT - 1],
                        start=False, stop=True,
                    )
                    ob = obpool.tile([P, B], BF16, name="ob", tag="ob", bufs=4)
                    if o % 2 == 0:
                        nc.scalar.copy(ob, psum_o[o])
                    else:
                        nc.vector.tensor_copy(ob, psum_o[o])
                    nc.gpsimd.dma_start(rs_in[o * P:(o + 1) * P, :], ob)

            # ---- ReduceScatter output currents; each core keeps its 128 rows ----
            rs_out = dramp.tile([O_SL, B], BF16, name="rs_out")
            nc.gpsimd.collective_compute(
                "ReduceScatter", AL.add,
                replica_groups=[list(range(NCORES))],
                ins=[rs_in[:, :]], outs=[rs_out[:, :]],
            )

            # ---- output layer: same scan + first-crossing, scaled by 0.1 ----
            ro = spool.tile([P, B], BF16, name="ro", tag="ro")
            nc.sync.dma_start(ro, rs_out[:, :])
            wo = spool.tile([P, B], F32, name="wo", tag="wscan")
            nc.vector.tensor_tensor_scan(wo, alpha_t, ro, 0.0, AL.mult, AL.add)
            g2 = spool.tile([P, B], F32, name="g2", tag="g")
            nc.vector.tensor_scalar(g2, wo, THR, None, AL.is_ge)
            midx2 = spool.tile([P, B], F32, name="midx2", tag="midx")
            nc.vector.scalar_tensor_tensor(
                midx2, g2, -BIGB, iota_hi, AL.mult, AL.add)
            bmin2 = spool.tile([P, 1], F32, name="bmin2", tag="bmin")
            nc.vector.tensor_reduce(
                bmin2, midx2, axis=mybir.AxisListType.X, op=AL.min)
            out_sb = spool.tile([P, B], F32, name="out_sb", tag="outsb")
            nc.vector.tensor_scalar(
                out_sb, iota_f, bmin2, float(np.float32(0.1)),
                AL.is_equal, AL.mult)
            nc.sync.dma_start(res[:, :], out_sb)

    nc.finalize()
    return nc


_STATE = {}


def _get_uniforms():
    """The key-42 uniform draws the reference's bernoulli uses — input-independent
    constants. [B, IN_DIM, T] float32, computed once on host."""
    if "u" not in _STATE:
        import jax
        import jax.numpy as jnp
        f = jax.jit(lambda: jax.random.uniform(
            jax.random.key(42), (B, IN_DIM, T), jnp.float32), backend="cpu")
        _STATE["u"] = np.asarray(f())
    return _STATE["u"]


def _get_nc():
    if "nc" not in _STATE:
        _STATE["nc"] = _build_nc()
    return _STATE["nc"]


def make_in_maps(x, W_ih, W_ho):
    import ml_dtypes

    x = np.ascontiguousarray(x, dtype=np.float32)
    W_ih = np.ascontiguousarray(W_ih, dtype=np.float32)
    W_ho = np.ascontiguousarray(W_ho, dtype=np.float32)
    u = _get_uniforms()

    in_maps = []
    for m in range(NCORES):
        isl = slice(m * IN_SL, (m + 1) * IN_SL)
        # u[b, i, t] -> [i_slice, t, b] -> [4, 128, T, B] bf16
        uc = np.ascontiguousarray(
            u[:, isl, :].transpose(1, 2, 0).reshape(IN_SL // P, P, T, B)
        ).astype(ml_dtypes.bfloat16)
        in_maps.append({
            "xt": np.ascontiguousarray(x[:, isl].T),
            "u": uc,
            "w_ih": np.ascontiguousarray(
                W_ih[:, m * H_SL:(m + 1) * H_SL]).astype(ml_dtypes.bfloat16),
            "w_ho": np.ascontiguousarray(
                W_ho[m * H_SL:(m + 1) * H_SL, :]).astype(ml_dtypes.bfloat16),
        })
    return in_maps


def assemble_out(results):
    out = np.empty((B, OUT), np.float32)
    for m in range(NCORES):
        out[:, m * O_SL:(m + 1) * O_SL] = results[m]["res"].T
    return out


def kernel(x, W_ih, W_ho):
    nc = _get_nc()
    in_maps = make_in_maps(x, W_ih, W_ho)
    r = run_bass_kernel_spmd(nc, in_maps, list(range(NCORES)))

    return assemble_out(r.results)


if __name__ == "__main__":
    # quick self-exercise with random inputs
    rng = np.random.default_rng(0)
    x = rng.standard_normal((B, IN_DIM), dtype=np.float32)
    W_ih = np.clip(0.5 + 0.1 * rng.standard_normal((IN_DIM, HID)), 0, 1).astype(np.float32)
    W_ho = np.clip(0.5 + 0.1 * rng.standard_normal((HID, OUT)), 0, 1).astype(np.float32)
    out = kernel(x, W_ih, W_ho)
    print("out", out.shape, out.dtype, "nonzero rows:", np.unique(np.nonzero(out)[0]))

